# revision 41
# baseline (speedup 1.0000x reference)
"""Trainium2 Bass kernel for nn_Encoder_ATTENTION (gnn_message_passing).

Math (per (b, n)):
  wn     = normalize(w_r_weight[rid[b,n]])            (table prep, host)
  d      = <e[b,n,:], wn>
  e_tr   = e - d * wn                                  (unmasked; mask folded into coeffs)
  h      = tanh(W @ [z_q[b]; e_tr] + bias)             (z-part via per-b zw table, on-chip)
  alpha  = u_a . h + u_a_b
  E      = exp(alpha) * (rid < CNT_E)
  attn   = E / sum_n(E) + rw                           (softmax w/o max-sub; logits are small)
  out[b] = sum_n (attn * mask) * e_tr

Sharding: data-parallel over batch, 512 batch rows per core x 8 cores.
Host does layout-only prep (dtype casts, transposes/padding of weights +
index tensors); all batch-dependent math runs on device.

v2 vs v1:
  - bf16 datapath everywhere (DVE 2x mode, matmuls at full rate, DMA halved)
  - wn-table gather via one batched gpsimd.dma_gather per 8 tiles
    (vs 8 indirect DMAs: swdge descriptor-gen time drops ~8x)
  - e_tr transposed for the h-matmul via XBAR dma_start_transpose
    (one DMA instr per 8 tiles) instead of PE transposes
  - DVE elementwise/reduce ops batched over 8 tiles per instruction
  - tanh/u-dot batched over 2-tile PSUM groups
"""

import sys

import numpy as np


def _ensure_path():
    for p in ("/opt/trn_rl_repo", "/root/.axon_site/_ro/trn_rl_repo"):
        if p not in sys.path:
            sys.path.append(p)


_ensure_path()

from contextlib import ExitStack

import ml_dtypes

import concourse.bacc as bacc
import concourse.bass as bass
import concourse.tile as tile
from concourse import mybir
from concourse.bass import IndirectOffsetOnAxis
from concourse.masks import make_identity

B, NB, DIM = 4096, 64, 256
H = 2 * DIM
NCORES = 8
BC = B // NCORES            # 512 batch rows per core
ROWS = BC * NB              # 32768 (b, n) rows per core
NT = ROWS // 128            # 256 tiles of 128 rows
TPB = 8                     # tiles per group (gather/softmax/output batching)
NBATCH = NT // TPB          # 32
CNT_E = 1000                # padding relation id
N_WR = CNT_E + 1            # 1001 rows in w_r table
N_ZQ = CNT_E                # 1000 rows in zq table
WN_PAD = 1024               # padded wn table rows

f32 = mybir.dt.float32
bf16 = mybir.dt.bfloat16
fp8 = mybir.dt.float8e4
i32 = mybir.dt.int32
i16 = mybir.dt.int16
AF = mybir.ActivationFunctionType
OP = mybir.AluOpType

BF = ml_dtypes.bfloat16
F8 = ml_dtypes.float8_e4m3


def build_nc(nbatch=NBATCH):
    nc = bacc.Bacc("TRN2")

    e_d = nc.dram_tensor("e", [ROWS, DIM], bf16, kind="ExternalInput")
    ridT_d = nc.dram_tensor("ridT", [128, NT], i32, kind="ExternalInput")
    rwT_d = nc.dram_tensor("rwT", [128, NT], f32, kind="ExternalInput")
    idx_d = nc.dram_tensor("idx", [128, ROWS // 16], i16, kind="ExternalInput")
    qoff_d = nc.dram_tensor("qoff", [128, BC // 128], i32, kind="ExternalInput")
    wn_d = nc.dram_tensor("wn", [WN_PAD, DIM], bf16, kind="ExternalInput")
    zq_d = nc.dram_tensor("zq", [N_ZQ, DIM], bf16, kind="ExternalInput")
    WT_d = nc.dram_tensor("WT", [128, 4, H], bf16, kind="ExternalInput")
    bias_d = nc.dram_tensor("bias", [1, H], f32, kind="ExternalInput")
    ua_d = nc.dram_tensor("ua", [1, H], f32, kind="ExternalInput")
    uab_d = nc.dram_tensor("uab", [1, 1], f32, kind="ExternalInput")
    out_d = nc.dram_tensor("out", [BC, DIM], f32, kind="ExternalOutput")

    with tile.TileContext(nc) as tc, ExitStack() as ctx:
        const = ctx.enter_context(tc.tile_pool(name="const", bufs=1))
        epool = ctx.enter_context(tc.tile_pool(name="epool", bufs=2))
        gpool = ctx.enter_context(tc.tile_pool(name="gpool", bufs=2))
        xpool = ctx.enter_context(tc.tile_pool(name="xpool", bufs=2))
        dgp = ctx.enter_context(tc.tile_pool(name="dgp", bufs=2))
        etrp = ctx.enter_context(tc.tile_pool(name="etrp", bufs=3))
        tpool = ctx.enter_context(tc.tile_pool(name="tpool", bufs=3))
        dpool = ctx.enter_context(tc.tile_pool(name="dpool", bufs=2))
        hpool = ctx.enter_context(tc.tile_pool(name="hpool", bufs=3))
        scp = ctx.enter_context(tc.tile_pool(name="scp", bufs=3))
        abp = ctx.enter_context(tc.tile_pool(name="abp", bufs=3))
        czp = ctx.enter_context(tc.tile_pool(name="czp", bufs=2))
        osp = ctx.enter_context(tc.tile_pool(name="osp", bufs=2))
        stp = ctx.enter_context(tc.tile_pool(name="stp", bufs=4))
        rsp = ctx.enter_context(tc.tile_pool(name="rsp", bufs=2))

        hps = ctx.enter_context(tc.tile_pool(name="hps", bufs=3, space="PSUM"))
        ops_ = ctx.enter_context(tc.tile_pool(name="ops", bufs=1, space="PSUM"))
        sps = ctx.enter_context(tc.tile_pool(name="sps", bufs=1, space="PSUM"))

        # ---------- constants ----------
        ident_f = const.tile([128, 128], f32)
        make_identity(nc, ident_f[:])
        ident = const.tile([128, 128], bf16)
        nc.vector.tensor_copy(ident[:], ident_f[:])
        # czmask[p, s, c] = 1.0 if c == 2s + p//64 else 0.0   [128, TPB, 16]
        czio = const.tile([128, TPB, 16], i32)
        nc.gpsimd.iota(czio[:], pattern=[[-2, TPB], [1, 16]], base=0, channel_multiplier=0)
        czf = const.tile([128, TPB, 16], f32)

        # blkpat[p, g] = 1.0 if p // 64 == g else 0.0          [128, 2]
        io2 = const.tile([128, 2], i32)
        nc.gpsimd.iota(io2[:], pattern=[[-64, 2]], base=0, channel_multiplier=1)
        bp0 = const.tile([128, 2], f32)
        bp1 = const.tile([128, 2], f32)
        nc.vector.tensor_scalar(out=bp0[:], in0=io2[:], scalar1=0, scalar2=None, op0=OP.is_ge)
        nc.vector.tensor_scalar(out=bp1[:], in0=io2[:], scalar1=63, scalar2=None, op0=OP.is_le)
        blkpat = const.tile([128, 2], f32)
        nc.vector.tensor_tensor(out=blkpat[:], in0=bp0[:], in1=bp1[:], op=OP.mult)
        nc.vector.tensor_scalar(
            out=czf[:], in0=czio[:], scalar1=blkpat[:, 1:2], scalar2=None, op0=OP.is_equal
        )
        czmask = const.tile([128, TPB, 16], bf16)
        nc.vector.tensor_copy(czmask[:], czf[:])

        # O2T[g, c] = 1.0 if c // 64 == g else 0.0             [2, 128]
        io3 = const.tile([2, 128], i32)
        nc.gpsimd.iota(io3[:], pattern=[[1, 128]], base=0, channel_multiplier=-64)
        ot0 = const.tile([2, 128], f32)
        ot1 = const.tile([2, 128], f32)
        nc.vector.tensor_scalar(out=ot0[:], in0=io3[:], scalar1=0, scalar2=None, op0=OP.is_ge)
        nc.vector.tensor_scalar(out=ot1[:], in0=io3[:], scalar1=63, scalar2=None, op0=OP.is_le)
        O2T = const.tile([2, 128], bf16)
        nc.vector.tensor_tensor(out=O2T[:], in0=ot0[:], in1=ot1[:], op=OP.mult)

        # O16[k, j, p] = 1.0 if k == 2j + p//64 else 0.0    [16, TPB, 128]
        o16a = const.tile([16, TPB, 128], i32)
        nc.gpsimd.iota(o16a[:], pattern=[[-2, TPB], [0, 128]], base=0, channel_multiplier=1)
        o16b = const.tile([16, TPB, 128], i32)
        nc.gpsimd.iota(o16b[:], pattern=[[0, TPB], [1, 2], [0, 64]], base=0, channel_multiplier=0)
        o16d = const.tile([16, TPB, 128], i32)
        nc.vector.tensor_tensor(out=o16d[:], in0=o16a[:], in1=o16b[:], op=OP.subtract)
        o16f = const.tile([16, TPB, 128], f32)
        nc.vector.tensor_scalar(out=o16f[:], in0=o16d[:], scalar1=0, scalar2=None, op0=OP.is_equal)
        O16 = const.tile([16, TPB, 128], bf16)
        nc.vector.tensor_copy(O16[:], o16f[:])

        # ---------- broadcast / table loads ----------
        # (partition-step-0 DMA broadcast crashes the exec unit on this
        # runtime; broadcast across partitions via a PE outer product instead)
        ones1 = const.tile([1, 128], bf16)
        nc.gpsimd.memset(ones1[:], 1.0)
        ua_row = const.tile([1, H], f32)
        nc.sync.dma_start(out=ua_row[:], in_=ua_d[:])
        ua_row_h = const.tile([1, H], bf16)
        nc.vector.tensor_copy(ua_row_h[:], ua_row[:])
        bias_row = const.tile([1, H], f32)
        nc.sync.dma_start(out=bias_row[:], in_=bias_d[:])
        bias_row_h = const.tile([1, H], bf16)
        nc.vector.tensor_copy(bias_row_h[:], bias_row[:])
        uab_row = const.tile([1, 1], f32)
        nc.sync.dma_start(out=uab_row[:], in_=uab_d[:])
        uab_row_h = const.tile([1, 1], bf16)
        nc.vector.tensor_copy(uab_row_h[:], uab_row[:])

        # u broadcast [128, 2, H] (for the 2-tile u-dot), bias broadcast [128, H]
        bc_ps = hps.tile([128, 2, H], f32, tag="hps")
        nc.tensor.matmul(out=bc_ps[:, 0, :], lhsT=ones1[:], rhs=ua_row_h[:])
        nc.tensor.matmul(out=bc_ps[:, 1, :], lhsT=ones1[:], rhs=bias_row_h[:])
        u_bc = const.tile([128, 2, H], bf16)
        nc.scalar.copy(u_bc[:, 0, :], bc_ps[:, 0, :])
        nc.scalar.copy(u_bc[:, 1, :], bc_ps[:, 0, :])
        biasb = const.tile([128, H], f32)
        nc.scalar.copy(biasb[:], bc_ps[:, 1, :])
        uab_b = const.tile([128, 1], f32)
        nc.gpsimd.partition_broadcast(uab_b[:], uab_row[:])

        WTs = const.tile([128, 4, H], bf16)
        nc.sync.dma_start(out=WTs[:], in_=WT_d[:])
        ridTs = const.tile([128, NT], i32)
        nc.sync.dma_start(out=ridTs[:], in_=ridT_d[:])
        rwTs = const.tile([128, NT], f32)
        nc.sync.dma_start(out=rwTs[:], in_=rwT_d[:])
        idx_sb = const.tile([128, ROWS // 16], i16)
        nc.sync.dma_start(out=idx_sb[:], in_=idx_d[:])
        qoffs = const.tile([128, BC // 128], i32)
        nc.sync.dma_start(out=qoffs[:], in_=qoff_d[:])

        # mask / masked rw, in tile-major layout [128, NT]
        ridTf = const.tile([128, NT], f32)
        nc.vector.tensor_copy(ridTf[:], ridTs[:])
        maskT = const.tile([128, NT], f32)
        nc.vector.tensor_scalar(out=maskT[:], in0=ridTf[:], scalar1=float(CNT_E), scalar2=None, op0=OP.is_lt)
        rwmT = const.tile([128, NT], f32)
        nc.vector.tensor_tensor(out=rwmT[:], in0=rwTs[:], in1=maskT[:], op=OP.mult)

        # ---------- zw table: zw[b] = W_z @ zq[q_rid[b]] + bias   [128, 4, H] ----------
        z_all = const.tile([128, BC // 128, DIM], bf16)
        for j in range(BC // 128):
            nc.gpsimd.indirect_dma_start(
                out=z_all[:, j, :],
                out_offset=None,
                in_=zq_d[:],
                in_offset=IndirectOffsetOnAxis(ap=qoffs[:, j : j + 1], axis=0),
            )
        zw_all = const.tile([128, BC // 128, H], bf16)
        for j in range(BC // 128):
            zT = tpool.tile([128, 2, 128], bf16, tag="zT")
            nc.sync.dma_start_transpose(out=zT[:], in_=z_all[:, j, :])
            zw_ps = hps.tile([128, 2, H], f32, tag="hps")
            for k in range(2):
                nc.tensor.matmul(
                    out=zw_ps[:, 0, :],
                    lhsT=zT[:, k, :],
                    rhs=WTs[:, k, :],
                    start=(k == 0),
                    stop=(k == 1),
                    skip_group_check=True,
                )
            nc.vector.tensor_tensor(out=zw_all[:, j, :], in0=zw_ps[:, 0, :], in1=biasb[:], op=OP.add)

        # zw_st[:, g, :]: the 16 batch rows of group g at partition base 0
        zw_st = const.tile([16, NBATCH, H], bf16)
        for g in range(NBATCH):
            g0 = (2 * TPB * g) % 128
            nc.sync.dma_start(
                out=zw_st[:, g, :], in_=zw_all[g0 : g0 + 16, (2 * TPB * g) // 128, :]
            )

        # ---------- main loop (3-stage software pipeline) ----------
        # proj(k):  e load, wn gather, d, e_tr, XBAR transpose, zw stage
        # attn(k):  h matmuls, tanh, u-dot  -> alpha
        # tail(k):  softmax, coeffs, output reduction, store
        # Issuing proj(k) before attn(k-1) before tail(k-2) keeps every
        # in-order engine queue free of head-of-line blocking.
        e_re = e_d[:].rearrange("(t p) d -> p t d", p=128)  # [128, NT, DIM]
        ipg = TPB * 128 // 16  # idx columns per group
        gstate = {}

        HT = TPB // 2  # tiles per half-chain

        def proj_phase(bt):
            t0 = bt * TPB
            e8 = epool.tile([128, TPB, DIM], bf16, tag="e8")
            nc.scalar.dma_start(out=e8[:], in_=e_re[:, t0 : t0 + TPB, :])

            G8 = gpool.tile([128, TPB, DIM], bf16, tag="G8")
            nc.gpsimd.dma_gather(
                G8[:], wn_d[:], idx_sb[:, ipg * bt : ipg * (bt + 1)],
                TPB * 128, TPB * 128, DIM,
            )

            X8 = xpool.tile([128, TPB, DIM], bf16, tag="X8")
            nc.vector.tensor_tensor(out=X8[:], in0=e8[:], in1=G8[:], op=OP.mult)
            d8 = dpool.tile([128, TPB], f32, tag="d8")
            nc.vector.tensor_reduce(out=d8[:], in_=X8[:], axis=mybir.AxisListType.X, op=OP.add)
            dG8 = dgp.tile([128, TPB, DIM], bf16, tag="dG8")
            for s in range(TPB):
                nc.scalar.activation(
                    out=dG8[:, s, :], in_=G8[:, s, :], func=AF.Copy,
                    scale=d8[:, s : s + 1],
                )
            etr8 = etrp.tile([128, TPB, DIM], bf16, tag="etr8")
            nc.vector.tensor_tensor(out=etr8[:], in0=e8[:], in1=dG8[:], op=OP.subtract)

            # eT8[:, 2*s + k, :] = (etr of tile s, dim chunk k) transposed
            eT8 = tpool.tile([128, 2 * TPB, 128], bf16, tag="eT8")
            nc.sync.dma_start_transpose(
                out=eT8[:], in_=etr8[:].rearrange("p a b -> p (a b)")
            )
            gstate[bt] = dict(etr8=etr8, eT8=eT8, bt=bt)

        def attn_phase(bt):
            st = gstate[bt]
            eT8 = st["eT8"]
            alpha_b = abp.tile([128, TPB], bf16, tag="alpha")
            for s in range(0, TPB, 2):
                h2 = hps.tile([128, 2, H], f32, tag="hps")
                for j in range(2):
                    nc.tensor.matmul(
                        out=h2[:, j, :], lhsT=eT8[:, 2 * (s + j), :], rhs=WTs[:, 2, :],
                        start=True, stop=False, skip_group_check=True,
                    )
                    nc.tensor.matmul(
                        out=h2[:, j, :], lhsT=eT8[:, 2 * (s + j) + 1, :], rhs=WTs[:, 3, :],
                        start=False, stop=False, skip_group_check=True,
                    )
                    nc.tensor.matmul(
                        out=h2[:, j, :], lhsT=O16[:, s + j, :], rhs=zw_st[:, bt, :],
                        start=False, stop=True, skip_group_check=True,
                    )

                h2s = hpool.tile([128, 2, H], bf16, tag="h")
                nc.scalar.activation(out=h2s[:], in_=h2[:], func=AF.Tanh)
                sc2 = scp.tile([128, 2, H], bf16, tag="sc")
                nc.vector.tensor_tensor(out=sc2[:], in0=h2s[:], in1=u_bc[:], op=OP.mult)
                with nc.allow_low_precision("bf16 attention logit is within tolerance"):
                    nc.vector.tensor_reduce(
                        out=alpha_b[:, s : s + 2], in_=sc2[:], axis=mybir.AxisListType.X, op=OP.add
                    )
            st["alpha_b"] = alpha_b

        def tail_phase(bt):
            st = gstate.pop(bt)
            t0 = bt * TPB
            alpha_b, etr8 = st["alpha_b"], st["etr8"]
            Eb = abp.tile([128, TPB], f32, tag="Eb")
            nc.scalar.activation(out=Eb[:], in_=alpha_b[:], func=AF.Exp, bias=uab_b[:, 0:1])
            Em = abp.tile([128, TPB], f32, tag="Em")
            nc.vector.tensor_tensor(out=Em[:], in0=Eb[:], in1=maskT[:, t0 : t0 + TPB], op=OP.mult)

            srb = sps.tile([128, 2 * TPB], f32, tag="sps")
            s_ps = srb[0:2, 0:TPB]
            nc.tensor.matmul(out=s_ps, lhsT=blkpat[:], rhs=Em[:])
            rS = rsp.tile([2, TPB], f32, tag="rS")
            nc.vector.reciprocal(rS[:], s_ps)
            rS_r = rsp.tile([2, TPB], bf16, tag="rSr")
            nc.vector.tensor_copy(rS_r[:], rS[:])
            rbc_ps = srb[:, TPB : 2 * TPB]
            nc.tensor.matmul(out=rbc_ps, lhsT=O2T[:], rhs=rS_r[:])

            coeff = abp.tile([128, TPB], f32, tag="coeff")
            nc.vector.tensor_tensor(out=coeff[:], in0=Em[:], in1=rbc_ps, op=OP.mult)
            nc.vector.tensor_tensor(out=coeff[:], in0=coeff[:], in1=rwmT[:, t0 : t0 + TPB], op=OP.add)

            # Cz: [128, TPB, 16]; block s has coeff at cols (2s, 2s+1), zeros elsewhere
            cz = czp.tile([128, TPB, 16], bf16, tag="cz")
            nc.vector.tensor_tensor(
                out=cz[:], in0=czmask[:], in1=coeff[:].to_broadcast((128, TPB, 16)), op=OP.mult
            )

            o_ps = ops_.tile([2 * TPB, DIM], f32, tag="ops")
            for s in range(TPB):
                nc.tensor.matmul(
                    out=o_ps[:],
                    lhsT=cz[:, s, :],
                    rhs=etr8[:, s, :],
                    start=(s == 0),
                    stop=(s == TPB - 1),
                    skip_group_check=True,
                )
            outS = osp.tile([2 * TPB, DIM], f32, tag="outS")
            nc.scalar.copy(outS[:], o_ps[:])
            nc.scalar.dma_start(out=out_d[2 * TPB * bt : 2 * TPB * (bt + 1), :], in_=outS[:])

        for k in range(nbatch + 2):
            if k < nbatch:
                proj_phase(k)
            if 1 <= k < nbatch + 1:
                attn_phase(k - 1)
            if k >= 2:
                tail_phase(k - 2)

    nc.finalize()
    return nc


_NC = None


def _get_nc():
    global _NC
    if _NC is None:
        _NC = build_nc()
    return _NC


def _prep_in_maps(inputs):
    e = np.asarray(inputs["batch_nei_e_emb"], dtype=np.float32).astype(BF)
    rid = np.asarray(inputs["batch_nei_rid"]).astype(np.int32)
    rw = np.asarray(inputs["batch_nei_rw"], dtype=np.float32)
    qr = np.asarray(inputs["batch_q_rid"]).astype(np.int32)

    w = np.asarray(inputs["w_r_weight"], dtype=np.float32)
    nrm = np.maximum(np.linalg.norm(w, axis=1, keepdims=True), 1e-12)
    wn = np.zeros((WN_PAD, DIM), BF)
    wn[:N_WR] = (w / nrm).astype(BF)
    WT = np.asarray(inputs["attn_W_w"], dtype=np.float32).T  # [in=512, out=512]
    WT4 = np.ascontiguousarray(WT.reshape(4, 128, H).transpose(1, 0, 2)).astype(BF)
    zq = np.ascontiguousarray(np.asarray(inputs["zq_weight"], dtype=np.float32)).astype(BF)
    bias = np.asarray(inputs["attn_W_b"], dtype=np.float32).reshape(1, H)
    ua = np.asarray(inputs["u_a_w"], dtype=np.float32).reshape(1, H)
    uab = np.asarray(inputs["u_a_b"], dtype=np.float32).reshape(1, 1)

    in_maps = []
    for c in range(NCORES):
        sl = slice(BC * c, BC * (c + 1))
        ec = np.ascontiguousarray(e[sl].reshape(ROWS, DIM))
        ridc = rid[sl].reshape(ROWS)
        rwc = rw[sl].reshape(ROWS)
        qc = qr[sl]
        rid_tp = ridc.reshape(NT, 128)  # [t, p] -> row t*128+p
        # dma_gather idx layout: group g covers rows [1024g, 1024(g+1));
        # index i lives at partition i%16, col i//16; 16-part block
        # replicated to all 128 partitions.
        idx16 = rid_tp.reshape(NBATCH, TPB * 128 // 16, 16).transpose(0, 2, 1)
        idx16 = idx16.reshape(NBATCH * 16, TPB * 128 // 16)  # [(g,16), ipg]
        idx = np.zeros((128, NBATCH * (TPB * 128 // 16)), np.int16)
        ipg = TPB * 128 // 16
        for g in range(NBATCH):
            blk = idx16[16 * g : 16 * (g + 1), :]  # [16, ipg]
            idx[:, ipg * g : ipg * (g + 1)] = np.tile(blk, (8, 1))
        in_maps.append(
            {
                "e": ec,
                "ridT": np.ascontiguousarray(rid_tp.T),
                "rwT": np.ascontiguousarray(rwc.reshape(NT, 128).T),
                "idx": idx,
                "qoff": np.ascontiguousarray(qc.reshape(BC // 128, 128).T),
                "wn": wn,
                "zq": zq,
                "WT": WT4,
                "bias": bias,
                "ua": ua,
                "uab": uab,
            }
        )
    return in_maps


def run_cores(inputs, trace=False, tmpdir=None):
    from concourse.bass_utils import run_bass_kernel_spmd

    nc = _get_nc()
    in_maps = _prep_in_maps(inputs)
    res = run_bass_kernel_spmd(
        nc, in_maps, core_ids=list(range(NCORES)), trace=trace, tmpdir=tmpdir
    )
    out = np.concatenate([res.results[c]["out"] for c in range(NCORES)], axis=0)
    return out, res


def kernel(**inputs):
    out, _ = run_cores(inputs, trace=False)
    return out
